# revision 12
# baseline (speedup 1.0000x reference)
"""Bilateral filter denoising (9x9 window) on 8 Trainium2 NeuronCores.

Full-input contract: kernel(noisy=[1,1,2048,2048] f32) -> [1,1,2048,2048] f32.

Strategy:
  - Shard H=2048 rows across 8 cores (256 rows each + 4-row halo), reflect
    padding and fp16 cast done host-side in numpy.
  - Per core, rows live in partitions, cols in the free dim. The 9 row
    shifts are 9 separately-DMA'd HBM->SBUF tiles; the 9 col shifts are
    free-dim AP offsets.
  - Per tap (di,dj): d = p - c (DVE fp16), s = d^2 (ACT Square or DVE mult),
    e = exp(-s/(2*sigma_i^2) + ln(sw)) (ACT, spatial weight folded into the
    bias), t = e*d (DVE fp16). Tap accumulation (sum e, sum e*d) runs on the
    otherwise-idle TensorEngine as identity matmuls accumulating into PSUM
    (f32), freeing the vector engine from 2 adds per tap.
  - out = clip(c + S/den, 0, 1), algebraically equal to the reference
    num/(den+eps) form up to ~1e-10 (den >= 1 so the eps is invisible in f32).
  - Odd column offsets would break the 4B alignment the DVE needs for its
    fp16 2x mode; those taps compute a by-one-column-shifted difference
    against an odd-base copy of the center row and compensate with a +1
    column offset on the matmul rhs.

Measured: max abs err 2.87e-4 vs the f32 reference on the real 8-core run;
TimelineSim cost model: ~517us per core (DVE 473us / ACT 467us / PE 282us
busy), fp16 diffs+weights with f32 PSUM accumulation.
"""

import numpy as np

WS = 9
PAD = 4
SIGMA_SPACE = 1.5
SIGMA_INT = 0.1
INV2SI2 = 1.0 / (2.0 * SIGMA_INT * SIGMA_INT)

H = 2048
W = 2048
N_CORES = 8
ROWS_PER_CORE = H // N_CORES  # 256
P = 128  # partitions


def _space_weight_np():
    ax = np.arange(-PAD, PAD + 1, dtype=np.float64)
    xx, yy = np.meshgrid(ax, ax, indexing="ij")
    return np.exp(-(xx**2 + yy**2) / (2.0 * SIGMA_SPACE**2))


def build_nc(rows, width, sq_dve_period=2, exact_recip=False, reps=1):
    """Build the per-core Bass program. rows must be a multiple of 128."""
    from contextlib import ExitStack

    import concourse.bacc as bacc
    import concourse.bass as bass  # noqa: F401
    import concourse.mybir as mybir
    import concourse.tile as tile

    dt = mybir.dt
    AF = mybir.ActivationFunctionType
    assert rows % P == 0
    n_tiles = rows // P
    wp = width + 2 * PAD
    CH = 512
    n_chunks = width // CH
    assert width % CH == 0

    sw = _space_weight_np()
    lnsw = np.log(sw)

    nc = bacc.Bacc("TRN2", target_bir_lowering=False)
    x16 = nc.dram_tensor("x16", [rows + 2 * PAD, wp], dt.float16, kind="ExternalInput")
    c32 = nc.dram_tensor("c32", [rows, width], dt.float32, kind="ExternalInput")
    ident = nc.dram_tensor("ident", [P, P], dt.float16, kind="ExternalInput")
    identn = nc.dram_tensor("identn", [P, P], dt.float16, kind="ExternalInput")
    out = nc.dram_tensor("out", [rows, width], dt.float32, kind="ExternalOutput")

    with ExitStack() as ctx:
        tc = ctx.enter_context(tile.TileContext(nc))
        ones = ctx.enter_context(tc.tile_pool(name="ones", bufs=1))
        rpool = ctx.enter_context(tc.tile_pool(name="rtiles", bufs=18))
        dpool = ctx.enter_context(tc.tile_pool(name="d", bufs=4))
        spool = ctx.enter_context(tc.tile_pool(name="s", bufs=4))
        epool = ctx.enter_context(tc.tile_pool(name="e", bufs=4))
        tpool = ctx.enter_context(tc.tile_pool(name="t", bufs=4))
        cpool = ctx.enter_context(tc.tile_pool(name="c", bufs=2))
        opool = ctx.enter_context(tc.tile_pool(name="o", bufs=2))
        small = ctx.enter_context(tc.tile_pool(name="small", bufs=4))
        den_pool = ctx.enter_context(tc.tile_pool(name="denp", bufs=4, space="PSUM"))
        s_pool = ctx.enter_context(tc.tile_pool(name="sp", bufs=4, space="PSUM"))

        id_t = ones.tile([P, P], dt.float16)
        nc.sync.dma_start(id_t[:], ident[:, :])
        id_n = ones.tile([P, P], dt.float16)
        nc.sync.dma_start(id_n[:], identn[:, :])
        ones16 = ones.tile([P, CH], dt.float16)
        nc.gpsimd.memset(ones16[:], 1.0)
        lnsw_t = ones.tile([P, WS * WS], dt.float32)
        for tt in range(WS * WS):
            nc.gpsimd.memset(lnsw_t[:, tt : tt + 1], float(lnsw[tt // WS, tt % WS]))

        for rep in range(reps):
          for b in range(n_tiles):
            rt = []
            for di in range(WS):
                t = rpool.tile([P, wp], dt.float16, tag="rt", name=f"rt{di}")
                nc.sync.dma_start(t[:], x16[b * P + di : b * P + di + P, :])
                rt.append(t)
            ct = cpool.tile([P, width], dt.float32)
            nc.sync.dma_start(ct[:], c32[b * P : (b + 1) * P, :])

            den_ps = [den_pool.tile([P, CH], dt.float32, tag="den", name=f"den{n}") for n in range(n_chunks)]
            s_ps = [s_pool.tile([P, CH], dt.float32, tag="S", name=f"S{n}") for n in range(n_chunks)]

            c16 = rt[PAD][:, PAD : PAD + width]
            # odd-base copy of the center row: c_odd[j] = rt4[j+1]. Lets the
            # odd-dj subtract read all three operands at even (4B-aligned)
            # fp16 offsets so the DVE keeps its 2x mode: we compute the
            # column-shifted difference u[c] = d[c-1] and compensate with a
            # +1 column offset on the matmul rhs below.
            c_odd = cpool.tile([P, wp - 4], dt.float16, tag="codd")
            nc.scalar.copy(c_odd[:], rt[PAD][:, 1 : wp - 3])
            for di in range(WS):
                for dj in range(WS):
                    tap = di * WS + dj
                    first = tap == 0
                    last = tap == WS * WS - 1
                    center_row = di == PAD
                    if center_row and dj < PAD:
                        # handled as the mirror of (PAD, 2*PAD - dj)
                        continue
                    if center_row and dj == PAD:
                        # center tap: e == 1, t == 0 -> den += 1 via a ones
                        # matmul, no S contribution
                        for n in range(n_chunks):
                            nc.tensor.matmul(
                                den_ps[n][:], id_t[:], ones16[:],
                                start=first, stop=last,
                            )
                        continue
                    pair = center_row and dj > PAD
                    o = dj - PAD
                    odd = dj % 2 == 1
                    if pair:
                        # compute e/t over the padded center range
                        # [-4..width-1] so the mirrored tap (PAD, PAD - o)
                        # becomes a shifted rhs read of the same tiles.
                        # e_tile[j] = value at center j - dir_off.
                        fd = width + 4
                        if odd:
                            in0 = rt[di][:, o + 1 : o + 1 + fd]
                            in1 = c_odd[:, 0:fd]
                            dir_off = 3
                        else:
                            in0 = rt[di][:, o : o + fd]
                            in1 = rt[PAD][:, 0:fd]
                            dir_off = 4
                        mir_off = dir_off - o
                    else:
                        fd = width + 2 if odd else width
                        dir_off = 1 if odd else 0
                        if odd:
                            in0 = rt[di][:, dj - 1 : dj - 1 + fd]
                            in1 = c_odd[:, 2 : 2 + fd]
                        else:
                            in0 = rt[di][:, dj : dj + fd]
                            in1 = c16
                    d = dpool.tile([P, width + 4], dt.float16, name="d")
                    nc.vector.tensor_sub(d[:, :fd], in0, in1)
                    s = spool.tile([P, width + 4], dt.float16, name="s")
                    if sq_dve_period and tap % sq_dve_period != 0:
                        nc.vector.tensor_mul(s[:, :fd], d[:, :fd], d[:, :fd])
                    else:
                        nc.scalar.activation(s[:, :fd], d[:, :fd], AF.Square)
                    e = epool.tile([P, width + 4], dt.float16, name="e")
                    nc.scalar.activation(
                        e[:, :fd], s[:, :fd], AF.Exp,
                        scale=-INV2SI2, bias=lnsw_t[:, tap : tap + 1]
                    )
                    t_ = tpool.tile([P, width + 4], dt.float16, name="t_")
                    nc.vector.tensor_mul(t_[:, :fd], e[:, :fd], d[:, :fd])
                    for n in range(n_chunks):
                        nc.tensor.matmul(
                            den_ps[n][:],
                            id_t[:],
                            e[:, dir_off + n * CH : dir_off + (n + 1) * CH],
                            start=first,
                            stop=last,
                        )
                        nc.tensor.matmul(
                            s_ps[n][:],
                            id_t[:],
                            t_[:, dir_off + n * CH : dir_off + (n + 1) * CH],
                            start=first,
                            stop=last,
                        )
                        if pair:
                            nc.tensor.matmul(
                                den_ps[n][:],
                                id_t[:],
                                e[:, mir_off + n * CH : mir_off + (n + 1) * CH],
                                start=False, stop=False,
                            )
                            nc.tensor.matmul(
                                s_ps[n][:],
                                id_n[:],
                                t_[:, mir_off + n * CH : mir_off + (n + 1) * CH],
                                start=False, stop=False,
                            )

            ot = opool.tile([P, width], dt.float32)
            for n in range(n_chunks):
                cs = slice(n * CH, (n + 1) * CH)
                rcp = small.tile([P, CH], dt.float32, tag="rcp")
                if exact_recip:
                    nc.vector.reciprocal(rcp[:], den_ps[n][:])
                else:
                    nc.vector.reciprocal_approx_fast(rcp[:], den_ps[n][:])
                u = small.tile([P, CH], dt.float32, tag="u")
                nc.vector.tensor_mul(u[:], s_ps[n][:], rcp[:])
                nc.vector.tensor_add(ot[:, cs], u[:], ct[:, cs])
            nc.vector.tensor_scalar(
                out=ot[:],
                in0=ot[:],
                scalar1=0.0,
                scalar2=1.0,
                op0=mybir.AluOpType.max,
                op1=mybir.AluOpType.min,
            )
            nc.sync.dma_start(out[b * P : (b + 1) * P, :], ot[:])
    nc.compile()
    return nc


def _prep_inputs(img, rows_per_core, n_cores):
    """img: [H, W] f32 -> list of per-core input dicts."""
    padded = np.pad(img, PAD, mode="reflect")
    ident = np.eye(P, dtype=np.float16)
    identn = (-np.eye(P)).astype(np.float16)
    in_maps = []
    for k in range(n_cores):
        r0 = k * rows_per_core
        x16 = np.ascontiguousarray(
            padded[r0 : r0 + rows_per_core + 2 * PAD, :]
        ).astype(np.float16)
        c32 = np.ascontiguousarray(img[r0 : r0 + rows_per_core, :])
        in_maps.append({"x16": x16, "c32": c32, "ident": ident, "identn": identn})
    return in_maps


TRACE = False
LAST_RESULTS = None


def kernel(noisy: np.ndarray) -> np.ndarray:
    global LAST_RESULTS
    from concourse.bass_utils import run_bass_kernel_spmd

    noisy = np.asarray(noisy)
    orig_shape = noisy.shape
    img = np.ascontiguousarray(noisy.reshape(H, W).astype(np.float32))

    nc = build_nc(ROWS_PER_CORE, W)
    in_maps = _prep_inputs(img, ROWS_PER_CORE, N_CORES)
    res = run_bass_kernel_spmd(
        nc, in_maps, core_ids=list(range(N_CORES)), trace=TRACE
    )
    LAST_RESULTS = res
    out = np.concatenate([r["out"] for r in res.results], axis=0)
    return out.reshape(orig_shape).astype(np.float32)

